# revision 17
# baseline (speedup 1.0000x reference)
"""DiffuseRouter kernel for 8 TRN2 NeuronCores.

Reference computation (enable_time=False, soft_time_routing=True):
    out[b, l, d] = (1/3) * sum_g sum_e expert_emb_g[e, b, l, d]
i.e. a uniform-weighted sum of 28 expert planes per batch element.

Sharding: pure data-parallel over batch B=8 -> one batch element per core.
No collectives needed (B == n_cores), which is strictly less traffic than
expert-parallel + all-reduce.

Precision: the host quantizes each plane to bf16 with the constant 1/3
granularity weight folded into the quantization scale (the probs are
input-independent: ones/3), so the device computes the pure 28-plane sum
and stores bf16 — 19.0 MB/core of HBM traffic instead of 38.0 MB/core.
Measured relative error ~6.9e-3, inside the 2e-2 gate.

Performance structure (the kernel is DMA-bandwidth-bound, ~410 GB/s/core):
- a single bf16 accumulation chain on DVE (bf16 tensor_tensor runs the 2x
  perf mode; RAW-1 chaining measured to run back-to-back), which needs no
  final chain-merge ops — DVE total (~39 us) sits under the ~44 us stream.
- the head loads as two half-pair tiles (planes 0|1 column halves) so the
  first DVE op fires ~2.5 us earlier than a full pair would allow.
- the body loads planes 2..23 as pair tiles [128, 2*2560] whose partition
  lines are 10240 B contiguous (full-size DMA descriptors, max per-SDMA-
  engine throughput).
- tile sizes taper at the stream end (singles for planes 24..26, quarters
  for plane 27) so the DVE backlog at end-of-stream shrinks to ~one small
  op: each final quarter is a single add (out.q = acc.q + p27.q) whose
  store issues immediately on the ACT ring (SP still drains input loads).
"""

import numpy as np
import ml_dtypes

import concourse.bacc as bacc
import concourse.tile as tile
from concourse import mybir
from concourse.alu_op_type import AluOpType
from concourse.bass_utils import run_bass_kernel_spmd

N_CORES = 8
E_TOTAL = 28  # 4 + 8 + 16 experts across the 3 granularity levels
L, D = 256, 1280
P = 128  # SBUF partitions
FD = (L // P) * D  # 2560 free-dim elements per partition per plane
H = FD // 2  # half of a plane's partition line
Q = FD // 4  # quarter of a plane's partition line
N_PAIRS = 11  # planes 2..23 as pair tiles
N_SING = 3  # planes 24..26 as single-plane tiles
SCALE = 1.0 / 3.0
BF16 = ml_dtypes.bfloat16

_NC_CACHE = None


def _build_nc():
    nc = bacc.Bacc(
        "TRN2", target_bir_lowering=False, debug=False, enable_partition_id=False
    )
    # Head: two half-pair tiles; tile j = (p0 half j | p1 half j).
    xh = nc.dram_tensor("xh", [2, P, FD], mybir.dt.bfloat16, kind="ExternalInput")
    xp = nc.dram_tensor(
        "xp", [N_PAIRS, P, 2 * FD], mybir.dt.bfloat16, kind="ExternalInput"
    )
    xs = nc.dram_tensor(
        "xs", [N_SING, P, FD], mybir.dt.bfloat16, kind="ExternalInput"
    )
    x27 = nc.dram_tensor("x27", [P, FD], mybir.dt.bfloat16, kind="ExternalInput")
    out = nc.dram_tensor("out", [L, D], mybir.dt.bfloat16, kind="ExternalOutput")

    out_t = out.ap().rearrange("(p a) d -> p (a d)", a=2)

    add = AluOpType.add
    LEFT = slice(0, FD)
    RIGHT = slice(FD, 2 * FD)

    with tile.TileContext(nc) as tc:
        with (
            tc.tile_pool(name="in", bufs=8) as pin,
            tc.tile_pool(name="acc", bufs=1) as pacc,
            tc.tile_pool(name="outp", bufs=1) as pout,
        ):
            acc = pacc.tile([P, FD], mybir.dt.bfloat16, name="acc", tag="acc")
            obuf = pout.tile([P, FD], mybir.dt.bfloat16, name="obuf", tag="obuf")

            # Head half-pairs initialize the chain per column half.
            for j in range(2):
                t = pin.tile([P, FD], mybir.dt.bfloat16, name=f"h{j}", tag=f"h{j}", bufs=1)
                nc.sync.dma_start(out=t[:], in_=xh.ap()[j])
                h = slice(j * H, (j + 1) * H)
                nc.vector.tensor_tensor(acc[:, h], t[:, :H], t[:, H:], add)
            # Body pairs (planes 2..21): two full-tile adds per pair.
            for k in range(N_PAIRS):
                t = pin.tile([P, 2 * FD], mybir.dt.bfloat16)
                nc.sync.dma_start(out=t[:], in_=xp.ap()[k])
                nc.vector.tensor_tensor(acc[:], acc[:], t[:, LEFT], add)
                nc.vector.tensor_tensor(acc[:], acc[:], t[:, RIGHT], add)
            # Tapered tail: single planes 22..26.
            for k in range(N_SING):
                t = pin.tile(
                    [P, FD], mybir.dt.bfloat16, name=f"s{k}", tag="sing", bufs=N_SING
                )
                nc.sync.dma_start(out=t[:], in_=xs.ap()[k])
                nc.vector.tensor_tensor(acc[:], acc[:], t[:], add)
            # Plane 27 quarters: one add each straight into the output tile,
            # store immediately (alternating ACT / SP rings — the SP ring's
            # input queue is empty by now).
            for qi in range(4):
                q = slice(qi * Q, (qi + 1) * Q)
                t = pin.tile(
                    [P, Q], mybir.dt.bfloat16, name=f"m{qi}", tag=f"m{qi}", bufs=1
                )
                nc.sync.dma_start(out=t[:], in_=x27.ap()[:, q])
                nc.vector.tensor_tensor(obuf[:, q], acc[:, q], t[:], add)
                nc.scalar.dma_start(out=out_t[:, q], in_=obuf[:, q])
    nc.compile()
    return nc


def _get_nc():
    global _NC_CACHE
    if _NC_CACHE is None:
        _NC_CACHE = _build_nc()
    return _NC_CACHE


def _run(inputs, trace=False, trace_kwargs=None):
    e0 = np.asarray(inputs["expert_emb_0"])
    e1 = np.asarray(inputs["expert_emb_1"])
    e2 = np.asarray(inputs["expert_emb_2"])
    B = e0.shape[1]
    assert B == N_CORES, f"expected B == {N_CORES}, got {B}"

    s = np.float32(SCALE)
    eb0 = (e0 * s).astype(BF16)
    eb1 = (e1 * s).astype(BF16)
    eb2 = (e2 * s).astype(BF16)

    in_maps = []
    for b in range(B):
        xb = np.concatenate([eb0[:, b], eb1[:, b], eb2[:, b]], axis=0)
        # [28, 256, 1280] -> partition lines: [28, 128, 2560]
        xl = xb.reshape(E_TOTAL, P, FD)
        # Head: [2, 128, 2560], tile j = (p0 half j | p1 half j).
        head = np.ascontiguousarray(
            xl[:2].reshape(2, P, 2, H).transpose(2, 1, 0, 3)
        ).reshape(2, P, FD)
        # Pair-interleave planes 2..21 -> 10240 B partition lines.
        pairs = np.ascontiguousarray(
            xl[2 : 2 + 2 * N_PAIRS].reshape(N_PAIRS, 2, P, FD).transpose(0, 2, 1, 3)
        ).reshape(N_PAIRS, P, 2 * FD)
        in_maps.append(
            {
                "xh": head,
                "xp": pairs,
                "xs": np.ascontiguousarray(xl[2 + 2 * N_PAIRS : 27]),
                "x27": np.ascontiguousarray(xl[27]),
            }
        )

    kw = {}
    if trace:
        kw["trace"] = True
        if trace_kwargs:
            kw.update(trace_kwargs)
    try:
        res = run_bass_kernel_spmd(_get_nc(), in_maps, list(range(N_CORES)), **kw)
    except Exception:
        # One retry: transient device errors (e.g. NRT unrecoverable after a
        # prior wedged run) usually clear on re-dispatch.
        res = run_bass_kernel_spmd(_get_nc(), in_maps, list(range(N_CORES)), **kw)
    out = np.stack([res.results[b]["out"] for b in range(B)], axis=0)
    return out.astype(np.float32), res


def kernel(**inputs) -> np.ndarray:
    out, _ = _run(inputs, trace=False)
    return out


# revision 19
# speedup vs baseline: 1.0051x; 1.0051x over previous
"""DiffuseRouter kernel for 8 TRN2 NeuronCores.

Reference computation (enable_time=False, soft_time_routing=True):
    out[b, l, d] = (1/3) * sum_g sum_e expert_emb_g[e, b, l, d]
i.e. a uniform-weighted sum of 28 expert planes per batch element.

Sharding: pure data-parallel over batch B=8 -> one batch element per core.
No collectives needed (B == n_cores), which is strictly less traffic than
expert-parallel + all-reduce.

Precision: the host quantizes each plane to bf16 with the constant 1/3
granularity weight folded into the quantization scale (the probs are
input-independent: ones/3), so the device computes the pure 28-plane sum
and stores bf16 — 19.0 MB/core of HBM traffic instead of 38.0 MB/core.
Measured relative error ~6.9e-3, inside the 2e-2 gate.

Performance structure (the kernel is DMA-bandwidth-bound, ~410 GB/s/core):
- a single bf16 accumulation chain on DVE (bf16 tensor_tensor runs the 2x
  perf mode; RAW-1 chaining measured to run back-to-back), which needs no
  final chain-merge ops — DVE total (~39 us) sits under the ~44 us stream.
- the head loads as two half-pair tiles (planes 0|1 column halves) so the
  first DVE op fires ~2.5 us earlier than a full pair would allow.
- the body loads planes 2..23 as pair tiles [128, 2*2560] whose partition
  lines are 10240 B contiguous (full-size DMA descriptors, max per-SDMA-
  engine throughput).
- tile sizes taper at the stream end (singles for planes 24..26, quarters
  for plane 27) so the DVE backlog at end-of-stream shrinks to ~one small
  op: each final quarter is a single add (out.q = acc.q + p27.q) whose
  store issues immediately on the ACT ring (SP still drains input loads).
"""

import numpy as np
import ml_dtypes

import concourse.bacc as bacc
import concourse.tile as tile
from concourse import mybir
from concourse.alu_op_type import AluOpType
from concourse.bass_utils import run_bass_kernel_spmd

N_CORES = 8
E_TOTAL = 28  # 4 + 8 + 16 experts across the 3 granularity levels
L, D = 256, 1280
P = 128  # SBUF partitions
FD = (L // P) * D  # 2560 free-dim elements per partition per plane
H = FD // 2  # half of a plane's partition line
Q = FD // 4  # quarter of a plane's partition line
N_PAIRS = 11  # planes 2..23 as pair tiles
N_SING = 3  # planes 24..26 as single-plane tiles
SCALE = 1.0 / 3.0
BF16 = ml_dtypes.bfloat16

_NC_CACHE = None


def _build_nc():
    nc = bacc.Bacc(
        "TRN2", target_bir_lowering=False, debug=False, enable_partition_id=False
    )
    # Head: two half-pair tiles; tile j = (p0 half j | p1 half j).
    xh = nc.dram_tensor("xh", [2, P, FD], mybir.dt.bfloat16, kind="ExternalInput")
    xp = nc.dram_tensor(
        "xp", [N_PAIRS, P, 2 * FD], mybir.dt.bfloat16, kind="ExternalInput"
    )
    xs = nc.dram_tensor(
        "xs", [N_SING, P, FD], mybir.dt.bfloat16, kind="ExternalInput"
    )
    x27 = nc.dram_tensor("x27", [P, FD], mybir.dt.bfloat16, kind="ExternalInput")
    out = nc.dram_tensor("out", [L, D], mybir.dt.bfloat16, kind="ExternalOutput")

    out_t = out.ap().rearrange("(p a) d -> p (a d)", a=2)

    add = AluOpType.add
    LEFT = slice(0, FD)
    RIGHT = slice(FD, 2 * FD)

    with tile.TileContext(nc) as tc:
        with (
            tc.tile_pool(name="in", bufs=8) as pin,
            tc.tile_pool(name="acc", bufs=1) as pacc,
            tc.tile_pool(name="outp", bufs=1) as pout,
        ):
            acc = pacc.tile([P, FD], mybir.dt.bfloat16, name="acc", tag="acc")
            obuf = pout.tile([P, FD], mybir.dt.bfloat16, name="obuf", tag="obuf")

            # Head half-pairs initialize the chain per column half.
            for j in range(2):
                t = pin.tile([P, FD], mybir.dt.bfloat16, name=f"h{j}", tag=f"h{j}", bufs=1)
                nc.sync.dma_start(out=t[:], in_=xh.ap()[j])
                h = slice(j * H, (j + 1) * H)
                nc.vector.tensor_tensor(acc[:, h], t[:, :H], t[:, H:], add)
            # Body pairs (planes 2..21): two full-tile adds per pair.
            for k in range(N_PAIRS):
                t = pin.tile([P, 2 * FD], mybir.dt.bfloat16)
                nc.sync.dma_start(out=t[:], in_=xp.ap()[k])
                nc.vector.tensor_tensor(acc[:], acc[:], t[:, LEFT], add)
                nc.vector.tensor_tensor(acc[:], acc[:], t[:, RIGHT], add)
            # Tapered tail: single planes 22..26.
            for k in range(N_SING):
                t = pin.tile(
                    [P, FD], mybir.dt.bfloat16, name=f"s{k}", tag="sing", bufs=N_SING
                )
                nc.sync.dma_start(out=t[:], in_=xs.ap()[k])
                nc.vector.tensor_tensor(acc[:], acc[:], t[:], add)
            # Plane 27 quarters: one add each straight into the output tile,
            # store immediately.  Store issues alternate the ACT and SP
            # engines so the ~0.6 us HWDGE descriptor-gen of consecutive
            # stores overlaps (by store time both rings' input queues have
            # drained; SP carries quarters 1,3 issued after its last load).
            for qi in range(4):
                q = slice(qi * Q, (qi + 1) * Q)
                t = pin.tile(
                    [P, Q], mybir.dt.bfloat16, name=f"m{qi}", tag=f"m{qi}", bufs=1
                )
                nc.sync.dma_start(out=t[:], in_=x27.ap()[:, q])
                nc.vector.tensor_tensor(obuf[:, q], acc[:, q], t[:], add)
                # Odd quarters (incl. the last) go to the ACT ring, which has
                # no pending packets; SP still drains the final input loads.
                eng = nc.sync if qi % 2 == 0 else nc.scalar
                eng.dma_start(out=out_t[:, q], in_=obuf[:, q])
    nc.compile()
    return nc


def _get_nc():
    global _NC_CACHE
    if _NC_CACHE is None:
        _NC_CACHE = _build_nc()
    return _NC_CACHE


def _run(inputs, trace=False, trace_kwargs=None):
    e0 = np.asarray(inputs["expert_emb_0"])
    e1 = np.asarray(inputs["expert_emb_1"])
    e2 = np.asarray(inputs["expert_emb_2"])
    B = e0.shape[1]
    assert B == N_CORES, f"expected B == {N_CORES}, got {B}"

    s = np.float32(SCALE)
    eb0 = (e0 * s).astype(BF16)
    eb1 = (e1 * s).astype(BF16)
    eb2 = (e2 * s).astype(BF16)

    in_maps = []
    for b in range(B):
        xb = np.concatenate([eb0[:, b], eb1[:, b], eb2[:, b]], axis=0)
        # [28, 256, 1280] -> partition lines: [28, 128, 2560]
        xl = xb.reshape(E_TOTAL, P, FD)
        # Head: [2, 128, 2560], tile j = (p0 half j | p1 half j).
        head = np.ascontiguousarray(
            xl[:2].reshape(2, P, 2, H).transpose(2, 1, 0, 3)
        ).reshape(2, P, FD)
        # Pair-interleave planes 2..21 -> 10240 B partition lines.
        pairs = np.ascontiguousarray(
            xl[2 : 2 + 2 * N_PAIRS].reshape(N_PAIRS, 2, P, FD).transpose(0, 2, 1, 3)
        ).reshape(N_PAIRS, P, 2 * FD)
        in_maps.append(
            {
                "xh": head,
                "xp": pairs,
                "xs": np.ascontiguousarray(xl[2 + 2 * N_PAIRS : 27]),
                "x27": np.ascontiguousarray(xl[27]),
            }
        )

    kw = {}
    if trace:
        kw["trace"] = True
        if trace_kwargs:
            kw.update(trace_kwargs)
    try:
        res = run_bass_kernel_spmd(_get_nc(), in_maps, list(range(N_CORES)), **kw)
    except Exception:
        # One retry: transient device errors (e.g. NRT unrecoverable after a
        # prior wedged run) usually clear on re-dispatch.
        res = run_bass_kernel_spmd(_get_nc(), in_maps, list(range(N_CORES)), **kw)
    out = np.stack([res.results[b]["out"] for b in range(B)], axis=0)
    return out.astype(np.float32), res


def kernel(**inputs) -> np.ndarray:
    out, _ = _run(inputs, trace=False)
    return out
